# revision 1
# baseline (speedup 1.0000x reference)
"""Trainium2 Bass kernel for a 3-layer GNN message-passing block.

Reference computation (per layer i):
    x1 = h @ Wfc[i] + bfc[i]                        # [N_SUB, D]
    x2 = scatter_mean(h, idx) @ Wsum[i] + bsum[i]   # [NUM_GRAPHS, D]
    h  = elu(x1 + x2[idx])
then
    out = relu(scatter_mean(h, idx) @ Wf1 + bf1) @ Wf2 + bf2

Strategy: data-parallel over 8 NeuronCores. The sorted graph index lets us
split graphs contiguously; each core owns 5 "windows" of <=128 graphs /
<=2560 subgraph rows (rows padded + permuted host-side so every window is
exactly 20 chunks of 128 rows). scatter_mean and the x2[idx] gather are both
expressed as one-hot matmuls on the TensorEngine, with the one-hot matrices
built host-side from the index (biases folded in via K=1 matmuls, 1/count
folded into the ACT copy that reads the scatter PSUM). All shapes/structure
are compile-time constants; the same program runs SPMD on all 8 cores with
per-core data, and the host re-assembles the [4096, 10] output.
"""

import numpy as np

P = 128
D = 512
N_SUB = 100000
NUM_GRAPHS = 4096
N_LAYERS = 3
NUM_TASKS = 10
N_CORES = 8
WIN_PER_CORE = 5
CH_PER_WIN = 20
ROWS_PER_WIN = CH_PER_WIN * P            # 2560
N_LOC = WIN_PER_CORE * ROWS_PER_WIN      # 12800 padded rows per core
CHUNKS = N_LOC // P                      # 100
G_WIN = P                                # graph slots per window
G_LOC = WIN_PER_CORE * G_WIN             # 640 graph slots per core
N_WIN_TOTAL = N_CORES * WIN_PER_CORE     # 40
DBLK = D // P                            # 4
D2 = 2 * D                               # 1024
D2BLK = D2 // P
PREFETCH = 6     # next-layer hT transposes prefetched (= stream pool bufs)                          # 8

_cached = {}


def _bf16():
    return np.float16


# ----------------------------------------------------------------- host prep

def _pack_windows(counts):
    """Split graphs 0..NUM_GRAPHS-1 into N_WIN_TOTAL contiguous windows with
    <= G_WIN graphs and <= ROWS_PER_WIN rows each, roughly row-balanced."""
    total = int(counts.sum())
    target = total / N_WIN_TOTAL
    wins = []
    g = 0
    rows_done = 0
    for w in range(N_WIN_TOTAL):
        g0 = g
        rows_w = 0
        while g < NUM_GRAPHS:
            c = int(counts[g])
            if rows_w + c > ROWS_PER_WIN or (g - g0) >= G_WIN:
                break
            if (w < N_WIN_TOTAL - 1 and rows_w > 0
                    and rows_done + rows_w + c > (w + 1) * target):
                remaining = total - (rows_done + rows_w)
                if remaining <= (N_WIN_TOTAL - w - 1) * ROWS_PER_WIN * 0.98:
                    break
            rows_w += c
            g += 1
        while g < NUM_GRAPHS and counts[g] == 0 and (g - g0) < G_WIN:
            g += 1
        rows_done += rows_w
        wins.append((g0, g))
    assert g == NUM_GRAPHS, f"window packing failed: {g}/{NUM_GRAPHS}"
    return wins


def _build_core_inputs(h, idx, counts, starts, wins, core, shared):
    bf16 = _bf16()
    h_pad = np.zeros((N_LOC, D), dtype=np.float32)
    slot = np.full(N_LOC, -1, dtype=np.int64)
    invc = np.zeros((P, WIN_PER_CORE), dtype=np.float32)  # [g_in_win, w]
    gmap = []
    for lw in range(WIN_PER_CORE):
        g0, g1 = wins[core * WIN_PER_CORE + lw]
        r0, r1 = int(starts[g0]), int(starts[g1])
        n = r1 - r0
        h_pad[lw * ROWS_PER_WIN: lw * ROWS_PER_WIN + n] = h[r0:r1]
        slot[lw * ROWS_PER_WIN: lw * ROWS_PER_WIN + n] = \
            lw * G_WIN + (idx[r0:r1] - g0)
        for j, g in enumerate(range(g0, g1)):
            invc[j, lw] = 1.0 / max(int(counts[g]), 1)
            gmap.append((g, lw * G_WIN + j))
    wsc = np.zeros((CHUNKS, P, P), dtype=np.float32)
    for c in range(CHUNKS):
        w = c // CH_PER_WIN
        s = slot[c * P:(c + 1) * P]
        real = np.nonzero(s >= 0)[0]
        wsc[c][real, s[real] - w * G_WIN] = 1.0
    wga = np.transpose(wsc, (0, 2, 1))
    # sbuf layout [P, CHUNKS*P]: partition p holds chunk-c block at cols 128c..
    wsc_flat = np.ascontiguousarray(
        np.transpose(wsc, (1, 0, 2)).reshape(P, CHUNKS * P)).astype(bf16)
    wga_flat = np.ascontiguousarray(
        np.transpose(wga, (1, 0, 2)).reshape(P, CHUNKS * P)).astype(bf16)
    h3 = h_pad.reshape(CHUNKS, P, DBLK, P)
    h16t = np.ascontiguousarray(h3.transpose(3, 0, 2, 1).reshape(P, CHUNKS * D))
    # layer-0 scatter_mean + x2 computed host-side (depends only on inputs)
    Wsum0, cbias0 = shared["_wsum0"], shared["_cbias0"]
    ssum = np.zeros((G_LOC, D), dtype=np.float32)
    np.add.at(ssum, slot[slot >= 0], h_pad[slot >= 0])
    mean0 = ssum * invc.T.reshape(G_LOC, 1)
    x2w0 = (mean0 @ Wsum0 + cbias0).astype(bf16)      # [G_LOC, D]
    x2w0b = np.ascontiguousarray(
        x2w0.reshape(WIN_PER_CORE, G_WIN, D).transpose(1, 0, 2)
        .reshape(G_WIN, WIN_PER_CORE * D))
    in_map = {
        "h16t": h16t.astype(bf16),
        "x2w0b": x2w0b,
        "wsc": wsc_flat,
        "wga": wga_flat,
        "invc": invc,
        **{k: v for k, v in shared.items() if not k.startswith("_")},
    }
    return in_map, gmap


def _prep_shared(Wfc, bfc, Wsum, bsum, Wf1, bf1, Wf2, bf2):
    bf16 = _bf16()
    # wfc/wsum packed [P, L*DBLK*D]: partition p, block (l,b) at cols (l*4+b)*D
    wfc = np.ascontiguousarray(
        Wfc.reshape(N_LAYERS, DBLK, P, D).transpose(2, 0, 1, 3)
        .reshape(P, N_LAYERS * DBLK * D)).astype(bf16)
    wsum = np.ascontiguousarray(
        Wsum.reshape(N_LAYERS, DBLK, P, D).transpose(2, 0, 1, 3)
        .reshape(P, N_LAYERS * DBLK * D)).astype(bf16)
    cbias = np.ascontiguousarray((bfc + bsum).reshape(1, N_LAYERS * D)).astype(bf16)
    # wf1 rhs blocks [b][half] = Wf1[b*P:(b+1)*P, half*D:(half+1)*D]
    wf1 = np.ascontiguousarray(
        Wf1.reshape(DBLK, P, 2, D).transpose(1, 0, 2, 3)
        .reshape(P, DBLK * D2)).astype(bf16)
    bf1w = np.ascontiguousarray(bf1.reshape(1, D2)).astype(bf16)
    # wf2 blocks [q] = Wf2[q*P:(q+1)*P, :]; packed [P, 8*NUM_TASKS]
    wf2 = np.ascontiguousarray(
        Wf2.reshape(D2BLK, P, NUM_TASKS).transpose(1, 0, 2)
        .reshape(P, D2BLK * NUM_TASKS)).astype(bf16)
    bf2w = np.ascontiguousarray(bf2.reshape(1, NUM_TASKS)).astype(bf16)
    return {
        "wfc": wfc, "wsum": wsum, "cbias": cbias,
        "wf1": wf1, "bf1w": bf1w, "wf2": wf2, "bf2w": bf2w,
        "_wsum0": Wsum[0].astype(np.float32),
        "_cbias0": (bfc[0] + bsum[0]).astype(np.float32)[None, :],
    }


# -------------------------------------------------------------- bass program

def _build_program():
    from contextlib import ExitStack

    import concourse.mybir as mybir
    import concourse.tile as tile
    from concourse import bacc

    bf = mybir.dt.float16
    f32 = mybir.dt.float32
    AF = mybir.ActivationFunctionType
    ALU = mybir.AluOpType

    nc = bacc.Bacc("TRN2", debug=False, target_bir_lowering=False,
                   num_devices=N_CORES, dynamic_dma_scratch_size=2048)

    # DRAM tensors (host pre-packs everything into few [P, X] tensors so each
    # loads with a single contiguous DMA)
    h16t_d = nc.dram_tensor("h16t", [P, CHUNKS * D], bf, kind="ExternalInput")
    x2w0_d = nc.dram_tensor("x2w0b", [G_WIN, WIN_PER_CORE * D], bf,
                            kind="ExternalInput")
    wsc_d = nc.dram_tensor("wsc", [P, CHUNKS * P], bf, kind="ExternalInput")
    wga_d = nc.dram_tensor("wga", [P, CHUNKS * P], bf, kind="ExternalInput")
    invc_d = nc.dram_tensor("invc", [P, WIN_PER_CORE], f32, kind="ExternalInput")
    wfc_d = nc.dram_tensor("wfc", [P, N_LAYERS * DBLK * D], bf, kind="ExternalInput")
    wsum_d = nc.dram_tensor("wsum", [P, N_LAYERS * DBLK * D], bf, kind="ExternalInput")
    cbias_d = nc.dram_tensor("cbias", [1, N_LAYERS * D], bf, kind="ExternalInput")
    wf1_d = nc.dram_tensor("wf1", [P, DBLK * D2BLK * P], bf, kind="ExternalInput")
    bf1_d = nc.dram_tensor("bf1w", [1, D2], bf, kind="ExternalInput")
    wf2_d = nc.dram_tensor("wf2", [P, D2BLK * NUM_TASKS], bf, kind="ExternalInput")
    bf2_d = nc.dram_tensor("bf2w", [1, NUM_TASKS], bf, kind="ExternalInput")
    out_d = nc.dram_tensor("out", [NUM_TASKS, G_LOC], f32, kind="ExternalOutput")

    with tile.TileContext(nc) as tc, ExitStack() as ctx:
        const = ctx.enter_context(tc.tile_pool(name="const", bufs=1))
        hpool = ctx.enter_context(tc.tile_pool(name="h", bufs=1))
        stream = ctx.enter_context(tc.tile_pool(name="stream", bufs=6))
        work = ctx.enter_context(tc.tile_pool(name="work", bufs=2))
        x2pool = ctx.enter_context(tc.tile_pool(name="x2", bufs=1))
        psum = ctx.enter_context(tc.tile_pool(name="psum", bufs=2, space="PSUM"))
        psx1 = ctx.enter_context(tc.tile_pool(name="psx1", bufs=4, space="PSUM"))

        # ---- constants (single DMA per tensor; no DMA slot reuse anywhere)
        ones = const.tile([1, P], bf, tag="ones")
        nc.vector.memset(ones[:], 1.0)
        x2w0_t = []
        for w in range(WIN_PER_CORE):
            t0w = x2pool.tile([P, D], bf, tag=f"x2w{w}", name=f"x2w0_{w}")
            nc.sync.dma_start(t0w[:], x2w0_d[:, w * D:(w + 1) * D])
            x2w0_t.append(t0w)
        WCH = CH_PER_WIN * P
        wgab_w = [const.tile([P, WCH], bf, tag=f"wgab{k}", name=f"wgab{k}")
                  for k in range(WIN_PER_CORE)]
        wscb_w = [const.tile([P, WCH], bf, tag=f"wscb{k}", name=f"wscb{k}")
                  for k in range(WIN_PER_CORE)]
        wfc_lt = [const.tile([P, DBLK * D], bf, tag=f"wfc{l}", name=f"wfc{l}")
                  for l in range(N_LAYERS)]
        nc.sync.dma_start(wfc_lt[0][:], wfc_d[:, :DBLK * D])
        nc.sync.dma_start(wgab_w[0][:], wga_d[:, 0 * WCH:1 * WCH])
        invc_t = const.tile([P, WIN_PER_CORE], f32, tag="invc")
        nc.sync.dma_start(invc_t[:], invc_d[:, :])
        # h tiles are written by layer-0 ELU (no load); hT for layer 0 is the
        # host-pretransposed copy, streamed progressively with the one-hots.
        h_t = [hpool.tile([P, D], bf, tag=f"h{c}", name=f"h{c}")
               for c in range(CHUNKS)]
        hT0 = []

        def load_hT0(c):
            ht = stream.tile([P, D], bf, tag="hT", name=f"hT0_{c}", bufs=8)
            nc.sync.dma_start(ht[:], h16t_d[:, c * D:(c + 1) * D])
            hT0.append(ht)

        for c in range(6):
            load_hT0(c)
        for k in range(WIN_PER_CORE):
            nc.sync.dma_start(wscb_w[k][:], wsc_d[:, k * WCH:(k + 1) * WCH])
            if k + 1 < WIN_PER_CORE:
                nc.sync.dma_start(wgab_w[k + 1][:],
                                  wga_d[:, (k + 1) * WCH:(k + 2) * WCH])
            for c in range(6 + k * 6, 6 + (k + 1) * 6):
                load_hT0(c)
        for c in range(36, CHUNKS):
            load_hT0(c)
        # remaining constants on the ACT HWDGE queue (wsumb/cbias first: the
        # layer-1 x2 windows need them ~55us in; wfc1/2 not until ~160us)
        wsumb = const.tile([P, N_LAYERS * DBLK * D], bf, tag="wsumb")
        nc.scalar.dma_start(wsumb[:], wsum_d[:, :])
        cbiasb = const.tile([1, N_LAYERS * D], bf, tag="cbiasb")
        nc.scalar.dma_start(cbiasb[:], cbias_d[:, :])
        nc.scalar.dma_start(wfc_lt[1][:], wfc_d[:, DBLK * D:2 * DBLK * D])
        nc.scalar.dma_start(wfc_lt[2][:], wfc_d[:, 2 * DBLK * D:])
        wf1b = const.tile([P, DBLK * D2BLK * P], bf, tag="wf1b")
        nc.scalar.dma_start(wf1b[:], wf1_d[:, :])
        bf1_t = const.tile([1, D2], bf, tag="bf1")
        nc.scalar.dma_start(bf1_t[:], bf1_d[:, :])
        wf2b = const.tile([P, D2BLK * NUM_TASKS], bf, tag="wf2b")
        nc.scalar.dma_start(wf2b[:], wf2_d[:, :])
        bf2_t = const.tile([1, NUM_TASKS], bf, tag="bf2")
        nc.scalar.dma_start(bf2_t[:], bf2_d[:, :])

        def wfc_s(layer, b):
            return wfc_lt[layer][:, b * D:(b + 1) * D]

        def wsum_s(layer, b):
            return wsumb[:, (layer * DBLK + b) * D:(layer * DBLK + b + 1) * D]

        def cbias_s(layer):
            return cbiasb[:, layer * D:(layer + 1) * D]

        def wf1_h(b, half):
            i = b * 2 + half
            return wf1b[:, i * D:(i + 1) * D]

        def wf2_s(q):
            return wf2b[:, q * NUM_TASKS:(q + 1) * NUM_TASKS]

        def wsc_c(c):
            return wscb_w[c // CH_PER_WIN][:, (c % CH_PER_WIN) * P:
                                           (c % CH_PER_WIN + 1) * P]

        def wga_c(c):
            return wgab_w[c // CH_PER_WIN][:, (c % CH_PER_WIN) * P:
                                           (c % CH_PER_WIN + 1) * P]

        hT_next = hT0

        def x2_window(meanT, w, layer):
            """x2 = meanT.T @ Wsum + (bfc+bsum), as bf16 [g, d]."""
            ps = psum.tile([P, D], f32, tag="x2")
            for b in range(DBLK):
                nc.tensor.matmul(ps[:], lhsT=meanT[:, b * P:(b + 1) * P],
                                 rhs=wsum_s(layer, b),
                                 start=(b == 0), stop=False)
            nc.tensor.matmul(ps[:], lhsT=ones[:, :P], rhs=cbias_s(layer),
                             start=False, stop=True)
            x2w = x2pool.tile([P, D], bf, tag=f"x2w{w}", name=f"x2w{w}")
            nc.scalar.activation(x2w[:], ps[:], AF.Copy)
            return x2w[:]

        # layer-0 x2 comes precomputed from the host
        x2ws = {w: x2w0_t[w][:] for w in range(WIN_PER_CORE)}

        out_sb = const.tile([NUM_TASKS, G_LOC], f32, tag="out")

        head_pend = {}

        def head_t(w, meanT):
            """t = relu(hg @ Wf1 + bf1), transposed; out-MMs deferred."""
            t = work.tile([P, D2], bf, tag="tT", bufs=1, name=f"t{w}")
            tTh = []
            for half in range(2):
                ps = psx1.tile([P, D], f32, tag="x1", name=f"hps{w}_{half}")
                for b in range(DBLK):
                    nc.tensor.matmul(ps[:], lhsT=meanT[:, b * P:(b + 1) * P],
                                     rhs=wf1_h(b, half),
                                     start=(b == 0), stop=False)
                nc.tensor.matmul(ps[:], lhsT=ones[:, :P],
                                 rhs=bf1_t[:, half * D:(half + 1) * D],
                                 start=False, stop=True)
                nc.scalar.activation(t[:, half * D:(half + 1) * D],
                                     ps[:], AF.Relu)
                th = work.tile([P, D], bf, tag=f"tTh{half}", bufs=2,
                               name=f"tTh{w}_{half}")
                nc.sync.dma_start(th[:].rearrange("p (b r) -> p b r", b=DBLK),
                                  t[:, half * D:(half + 1) * D],
                                  transpose=True)
                tTh.append(th)
            head_pend[w] = tTh

        def head_out(w):
            tTh = head_pend.pop(w)
            pso = psum.tile([NUM_TASKS, P], f32, tag="x2", name=f"pso{w}")
            for q in range(D2BLK):
                nc.tensor.matmul(pso[:], lhsT=wf2_s(q),
                                 rhs=tTh[q // 4][:, (q % 4) * P:
                                                 (q % 4 + 1) * P],
                                 start=(q == 0), stop=False)
            nc.tensor.matmul(pso[:], lhsT=bf2_t[:], rhs=ones[:, :P],
                             start=False, stop=True)
            nc.vector.tensor_copy(out_sb[:, w * P:(w + 1) * P], pso[:])

        def head_window(w, meanT):
            head_t(w, meanT)

        # ---- update passes; each folds the NEXT context's scatter (layer
        # l+1's, or the head's) in with a small lag so the PE never has a
        # serial scatter phase after layer 0.
        LAG = 4
        for layer in range(N_LAYERS):
            hTts = hT_next
            for c in range(len(hTts), CHUNKS):
                hTt = stream.tile([P, D], bf, tag="hT", name=f"hT{c}", bufs=8)
                nc.sync.dma_start(hTt[:].rearrange("p (b r) -> p b r", b=DBLK),
                                  h_t[c][:], transpose=True)
                hTts.append(hTt)
            hT_next = []
            nxt_x2ws = {}
            sc_state = {}
            pend = {}

            def emit_next_scatter(c, layer=layer, sc_state=sc_state,
                                  pend=pend):
                i = c % CH_PER_WIN
                w = c // CH_PER_WIN
                if i == 0:
                    sc_state["ps"] = psum.tile([P, D], f32, tag="sc",
                                               name=f"sc{layer}_{w}")
                nc.tensor.matmul(sc_state["ps"][:], lhsT=wsc_c(c),
                                 rhs=h_t[c][:],
                                 start=(i == 0), stop=(i == CH_PER_WIN - 1))
                if i == CH_PER_WIN - 1:
                    mean = work.tile([P, D], bf, tag="mean", bufs=1,
                                     name=f"mean{layer}_{w}")
                    nc.scalar.activation(mean[:], sc_state["ps"][:], AF.Copy,
                                         scale=invc_t[:, w:w + 1])
                    meanT = work.tile([P, D], bf, tag="meanT", bufs=1,
                                      name=f"meanT{layer}_{w}")
                    nc.sync.dma_start(
                        meanT[:].rearrange("p (b r) -> p b r", b=DBLK),
                        mean[:], transpose=True)
                    pend[w] = meanT

            def emit_window_tail(w, layer=layer, nxt_x2ws=nxt_x2ws,
                                 pend=pend):
                meanT = pend.pop(w)
                if layer < N_LAYERS - 1:
                    nxt_x2ws[w] = x2_window(meanT, w, layer + 1)
                else:
                    head_window(w, meanT)

            for c in range(CHUNKS):
                w = c // CH_PER_WIN
                hTt = hTts[c]
                ps = psx1.tile([P, D], f32, tag="x1")
                for b in range(DBLK):
                    nc.tensor.matmul(ps[:], lhsT=hTt[:, b * P:(b + 1) * P],
                                     rhs=wfc_s(layer, b),
                                     start=(b == 0), stop=False)
                nc.tensor.matmul(ps[:], lhsT=wga_c(c), rhs=x2ws[w],
                                 start=False, stop=True)
                # ELU: h = relu(z) + (min(exp(z), 1) - 1)
                e = work.tile([P, D], bf, tag="e")
                nc.scalar.activation(e[:], ps[:], AF.Exp)
                me = work.tile([P, D], bf, tag="me")
                nc.vector.tensor_scalar(me[:], e[:], 1.0, -1.0,
                                        op0=ALU.min, op1=ALU.add)
                nc.vector.tensor_scalar(h_t[c][:], ps[:], 0.0, None,
                                        op0=ALU.max)
                nc.vector.tensor_tensor(h_t[c][:], h_t[c][:], me[:],
                                        op=ALU.add)
                if layer < N_LAYERS - 1 and c < PREFETCH:
                    nx = stream.tile([P, D], bf, tag="hTp", name=f"hTp{c}")
                    nc.sync.dma_start(
                        nx[:].rearrange("p (b r) -> p b r", b=DBLK),
                        h_t[c][:], transpose=True)
                    hT_next.append(nx)
                if c >= LAG:
                    emit_next_scatter(c - LAG)
                cw = (c - LAG - 8) // CH_PER_WIN     # window whose meanT has
                if c >= LAG + 8 and (c - LAG - 8) % CH_PER_WIN == CH_PER_WIN - 1:
                    emit_window_tail(cw)             # had 8 chunks to settle
                cw2 = (c - LAG - 14) // CH_PER_WIN
                if (c >= LAG + 14
                        and (c - LAG - 14) % CH_PER_WIN == CH_PER_WIN - 1
                        and cw2 in head_pend):
                    head_out(cw2)
            for c in range(CHUNKS - LAG, CHUNKS):
                emit_next_scatter(c)
            for w in sorted(pend):
                emit_window_tail(w)
            for w in sorted(head_pend):
                head_out(w)
            x2ws = nxt_x2ws

        nc.sync.dma_start(out_d[:, :], out_sb[:])

    nc.compile()
    return nc


# ------------------------------------------------------------------- kernel

def kernel(**inputs):
    h = np.asarray(inputs["h_subgraph"], dtype=np.float32)
    idx = np.asarray(inputs["subgraph_idx_batch"]).astype(np.int64)
    if not np.all(idx[:-1] <= idx[1:]):        # defensive: index must be sorted
        order = np.argsort(idx, kind="stable")
        h, idx = h[order], idx[order]

    counts = np.bincount(idx, minlength=NUM_GRAPHS)
    starts = np.concatenate([[0], np.cumsum(counts)])
    wins = _pack_windows(counts)
    shared = _prep_shared(
        np.asarray(inputs["Wfc"], np.float32), np.asarray(inputs["bfc"], np.float32),
        np.asarray(inputs["Wsum"], np.float32), np.asarray(inputs["bsum"], np.float32),
        np.asarray(inputs["Wf1"], np.float32), np.asarray(inputs["bf1"], np.float32),
        np.asarray(inputs["Wf2"], np.float32), np.asarray(inputs["bf2"], np.float32),
    )

    in_maps = []
    gmaps = []
    for core in range(N_CORES):
        m, gm = _build_core_inputs(h, idx, counts, starts, wins, core, shared)
        in_maps.append(m)
        gmaps.append(gm)

    _cached["in_maps"] = in_maps
    if "nc" not in _cached:
        _cached["nc"] = _build_program()
    nc = _cached["nc"]

    from concourse import bass_utils
    res = bass_utils.run_bass_kernel_spmd(
        nc, in_maps, core_ids=list(range(N_CORES)))

    out = np.zeros((NUM_GRAPHS, NUM_TASKS), dtype=np.float32)
    for core in range(N_CORES):
        o = res.results[core]["out"]           # [10, 640]
        for g, s in gmaps[core]:
            out[g] = o[:, s]
    return out

